# revision 1
# baseline (speedup 1.0000x reference)
"""Causal attention with ALiBi for nn_CausalAttention (B=4, T=2048, C=1024,
16 heads) on 8 TRN2 NeuronCores.

Sharding: batch (4) x head-group (2 groups of 8 heads) -> 8 cores.
Each core computes, for its batch b and head group g:
  qT/kT = (Wg.T @ x.T) projections in [d, t] layout, v in [t, d] layout,
  per head: sT[j, i] = qk/8 + slope*(j - i) via an augmented one-hot bias
  matmul (per-column -slope*i, numerically cancels in softmax) plus an ACT
  exp bias of +slope*j (exact fp32); causal masking by gpsimd affine_select
  (also kills Inf from masked overflow); PV with an appended ones column
  gives the softmax denominator; normalization via vector.reciprocal +
  gpsimd partition_broadcast; final y_partial = oT.T @ Wo_rows.
Host sums the two head-group partials per batch.

Matmuls run in float32r (TF32-like, ~1e-3 rel); probs/V in bf16.
"""

import math

import numpy as np

import concourse.bass as bass
import concourse.mybir as mybir
import concourse.tile as tile
from concourse import bacc
from concourse.bass_utils import run_bass_kernel_spmd

B, T, C = 4, 2048, 1024
NH, HD = 16, 64
NHC = 8  # heads per core
BLOCK_SIZE = 2048
NJB = T // 128  # 16 j-blocks
NCH = T // 512  # 4 i-chunks
P = 128

f32 = mybir.dt.float32
f32r = mybir.dt.float32r
bf16 = mybir.dt.bfloat16

LAST_RESULTS = None
_NC_CACHE = None


def get_slopes(n):
    def pow2(n):
        start = 2 ** (-(2 ** (-(math.log2(n) - 3))))
        return [start * start**i for i in range(n)]

    if math.log2(n).is_integer():
        return pow2(n)
    c = 2 ** math.floor(math.log2(n))
    return pow2(c) + get_slopes(2 * c)[0::2][: n - c]


# compact pT tile index: tiles (jb, c) with c >= jb//4
_PT_OFFS = []
_o = 0
for _jb in range(NJB):
    _PT_OFFS.append(_o)
    _o += NCH - _jb // 4
NPT = _o  # 40


def build_kernel():
    nc = bacc.Bacc("TRN2", target_bir_lowering=False, debug=False, num_devices=8)

    xT_d = nc.dram_tensor("xT", [C, T], f32, kind="ExternalInput").ap()
    wq_d = nc.dram_tensor("wq", [C, 512], f32, kind="ExternalInput").ap()
    wk_d = nc.dram_tensor("wk", [C, 512], f32, kind="ExternalInput").ap()
    wv_d = nc.dram_tensor("wv", [C, 512], f32, kind="ExternalInput").ap()
    wo_d = nc.dram_tensor("wo", [512, C], f32, kind="ExternalInput").ap()
    qaug_d = nc.dram_tensor("qaugb", [8, NHC, T], bf16, kind="ExternalInput").ap()
    kaug_d = nc.dram_tensor("kaugb", [8, NHC, T], bf16, kind="ExternalInput").ap()
    biasj_d = nc.dram_tensor("biasj", [P, NHC, NJB], f32, kind="ExternalInput").ap()
    y_d = nc.dram_tensor("y", [T, C], f32, kind="ExternalOutput").ap()

    xT_r = xT_d.rearrange("(cb p) t -> p cb t", p=P)  # [128, 8, 2048]
    wq_r = wq_d.rearrange("(cb p) m -> p cb m", p=P)  # [128, 8, 512]
    wk_r = wk_d.rearrange("(cb p) m -> p cb m", p=P)
    wv_r = wv_d.rearrange("(cb p) m -> p cb m", p=P)
    wo_r = wo_d.rearrange("(mb p) n -> p mb n", p=P)  # [128, 4, 1024]
    y_r = y_d.rearrange("(tb p) c -> p tb c", p=P)  # [128, 16, 1024]

    with tile.TileContext(nc) as tc:
        with (
            tc.tile_pool(name="persist", bufs=1) as persist,
            tc.tile_pool(name="work", bufs=2) as work,
            tc.tile_pool(name="psA", bufs=2, space="PSUM") as psA,
            tc.tile_pool(name="psB", bufs=2, space="PSUM") as psB,
            tc.tile_pool(name="psC", bufs=2, space="PSUM") as psC,
        ):
            # ---- persistent tiles ----
            # qT2/kT2: per head h, rows 0-63 = head data (d), rows 64-71 =
            # augmented bias rows; K=72 matmul contracts both at once.
            qT2 = persist.tile([72, NHC, T], bf16)
            kT2 = persist.tile([72, NHC, T], bf16)
            vaug = persist.tile([P, NJB, NHC, 66], bf16)
            oT = persist.tile([P, 4, T], bf16)
            biasj = persist.tile([P, NHC, NJB], f32)

            nc.gpsimd.memset(vaug[:, :, :, 64:66], 1.0)
            nc.sync.dma_start(biasj[:], biasj_d[:])
            # aug rows: kT2 row 64+r of head h is 1.0 iff r == h;
            # qT2 row 64+r of every head = -slope_r * i
            nc.sync.dma_start(kT2[64:72, :, :], kaug_d[:])
            nc.sync.dma_start(qT2[64:72, :, :], qaug_d[:])

            # ---- fused projections + attention ----
            # v first (vaug must be complete before the first PV); then per
            # head-pair m: project q/k for pair m, then emit QK/PV for its
            # heads, software-pipelined with lag 1 so the exp/select tail of
            # each head hides under the next head's work.
            wst_cm = tc.tile_pool(name="wst", bufs=2)
            wst = wst_cm.__enter__()
            xp1_cm = tc.tile_pool(name="xp1", bufs=2)
            xp1 = xp1_cm.__enter__()
            xr1_cm = tc.tile_pool(name="xr1", bufs=9)
            xr1 = xr1_cm.__enter__()
            wqk_cm = tc.tile_pool(name="wqk", bufs=1)
            wqk = wqk_cm.__enter__()

            def load_x_chunk(tck):
                xts = []
                for c in range(8):
                    x32 = xp1.tile([P, 512], f32, tag="x32")
                    nc.sync.dma_start(x32[:], xT_r[:, c, bass.ts(tck, 512)])
                    xtr = xr1.tile([P, 512], bf16, tag="xtr")
                    nc.vector.tensor_copy(xtr[:], x32[:])
                    xts.append(xtr)
                return xts

            # --- v projection ---
            with (
                tc.tile_pool(name="wvp", bufs=1) as wvp,
                tc.tile_pool(name="wvst", bufs=2) as wvst,
            ):
                wvr = wvp.tile([P, 8, 512], bf16)
                for c in range(8):
                    w32 = wvst.tile([P, 512], f32, tag="w32")
                    nc.sync.dma_start(w32[:], wv_r[:, c, :])
                    nc.vector.tensor_copy(wvr[:, c, :], w32[:])
                for tck in range(NCH):
                    xts = load_x_chunk(tck)
                    for tb in range(4):
                        psv = psB.tile([P, 512], f32, tag="pb")
                        for c in range(8):
                            nc.tensor.matmul(
                                psv[:],
                                xts[c][:, bass.ts(tb, P)],
                                wvr[:, c, :],
                                start=(c == 0),
                                stop=(c == 7),
                            )
                        nc.vector.tensor_copy(
                            vaug[:, 4 * tck + tb, :, 0:64],
                            psv[:].rearrange("p (h d) -> p h d", h=NHC),
                        )

            def project_pair(m):
                # load this pair's weight columns (bf16 chunks)
                wqm = wqk.tile([P, 8, P], bf16, tag="wqm")
                wkm = wqk.tile([P, 8, P], bf16, tag="wkm")
                for c in range(8):
                    wq32 = wst.tile([P, P], f32, tag="wc32")
                    nc.sync.dma_start(wq32[:], wq_r[:, c, bass.ts(m, P)])
                    nc.vector.tensor_copy(wqm[:, c, :], wq32[:])
                    wk32 = wst.tile([P, P], f32, tag="wc32")
                    nc.sync.dma_start(wk32[:], wk_r[:, c, bass.ts(m, P)])
                    nc.vector.tensor_copy(wkm[:, c, :], wk32[:])
                for tck in range(NCH):
                    xts = load_x_chunk(tck)
                    psq = psB.tile([P, 512], f32, tag="pb")
                    psk = psB.tile([P, 512], f32, tag="pb")
                    for c in range(8):
                        nc.tensor.matmul(
                            psq[:],
                            wqm[:, c, :],
                            xts[c][:],
                            start=(c == 0),
                            stop=(c == 7),
                        )
                        nc.tensor.matmul(
                            psk[:],
                            wkm[:, c, :],
                            xts[c][:],
                            start=(c == 0),
                            stop=(c == 7),
                        )
                    qstag = work.tile([P, 512], bf16, tag="qkstag")
                    kstag = work.tile([P, 512], bf16, tag="qkstag")
                    nc.vector.tensor_copy(qstag[:], psq[:])
                    nc.vector.tensor_copy(kstag[:], psk[:])
                    tsl = bass.ts(tck, 512)
                    nc.sync.dma_start(qT2[0:64, 2 * m, tsl], qstag[0:64, :])
                    nc.sync.dma_start(qT2[0:64, 2 * m + 1, tsl], qstag[64:128, :])
                    nc.sync.dma_start(kT2[0:64, 2 * m, tsl], kstag[0:64, :])
                    nc.sync.dma_start(kT2[0:64, 2 * m + 1, tsl], kstag[64:128, :])

            ptp_cm = tc.tile_pool(name="ptp", bufs=2)
            ptp = ptp_cm.__enter__()
            pT_of = {}

            def emit_qk(h):
                pT = ptp.tile([P, NPT, 512], bf16, tag="pT")
                pT_of[h] = pT
                for jb in range(NJB):
                    c0 = jb // 4
                    nact = NCH - c0
                    idx0 = _PT_OFFS[jb]
                    # sub-groups of <=2 chunks so QK can run ahead of exp
                    for g0 in range(0, nact, 2):
                        ng = min(2, nact - g0)
                        ssum = psA.tile([P, 2, 512], f32, tag="ssum")
                        for ci in range(ng):
                            c = c0 + g0 + ci
                            nc.tensor.matmul(
                                ssum[:, ci, :],
                                kT2[:, h, bass.ts(jb, P)],
                                qT2[:, h, bass.ts(c, 512)],
                                start=True,
                                stop=True,
                            )
                        nc.scalar.activation(
                            pT[:, idx0 + g0 : idx0 + g0 + ng, :],
                            ssum[:, 0:ng, :],
                            mybir.ActivationFunctionType.Exp,
                            bias=biasj[:, h, jb : jb + 1],
                            scale=1.0,
                        )
                    # causal mask on the diagonal tile (c == c0):
                    # keep where i - j >= 0 ; i = 512*c0 + f, j = 128*jb + p
                    nc.gpsimd.affine_select(
                        pT[:, idx0, :],
                        pT[:, idx0, :],
                        pattern=[[1, 512]],
                        compare_op=mybir.AluOpType.is_ge,
                        fill=0.0,
                        base=512 * c0 - 128 * jb,
                        channel_multiplier=-1,
                    )

            def emit_pv(h):
                hp = (h % 2) * 64
                hm = h // 2
                pT = pT_of.pop(h)
                for c in range(NCH):
                    pot = psC.tile([65, 512], f32, tag="pot")
                    njb = 4 * c + 4
                    for jb in range(njb):
                        nc.tensor.matmul(
                            pot[:],
                            vaug[:, jb, h, 0:65],
                            pT[:, _PT_OFFS[jb] + (c - jb // 4), :],
                            start=(jb == 0),
                            stop=(jb == njb - 1),
                        )
                    # copy out fast to release the PSUM bank, then normalize
                    # off the PV critical path.
                    potsb = work.tile([65, 512], f32, tag="potsb")
                    nc.vector.tensor_copy(potsb[:], pot[:])
                    # spread the 512 rowsums across 128 partitions so the
                    # reciprocal uses all DVE lanes (26ns vs 3.3us)
                    rs128 = work.tile([P, 4], f32, tag="rs128")
                    nc.sync.dma_start(rs128[:], potsb[64:65, :])
                    nc.vector.reciprocal(rs128[:], rs128[:])
                    srecip = persist.tile([1, 512], f32, tag="srecip")
                    nc.sync.dma_start(srecip[:], rs128[:])
                    bcast = persist.tile([64, 512], f32, tag="bcast")
                    nc.gpsimd.partition_broadcast(bcast[:], srecip[:])
                    nc.vector.tensor_tensor(
                        oT[hp : hp + 64, hm, bass.ts(c, 512)],
                        potsb[0:64, :],
                        bcast[:],
                        mybir.AluOpType.mult,
                    )

            for m in range(4):
                project_pair(m)
                emit_qk(2 * m)
                if m > 0:
                    emit_pv(2 * m - 1)
                emit_qk(2 * m + 1)
                emit_pv(2 * m)
            emit_pv(NHC - 1)

            ptp_cm.__exit__(None, None, None)
            wqk_cm.__exit__(None, None, None)
            xr1_cm.__exit__(None, None, None)
            xp1_cm.__exit__(None, None, None)
            wst_cm.__exit__(None, None, None)

            # ---- output projection ----
            with (
                tc.tile_pool(name="wop", bufs=1) as wop,
                tc.tile_pool(name="wst3", bufs=2) as wst3,
                tc.tile_pool(name="ypool", bufs=2) as ypool,
            ):
                wor = wop.tile([P, 4, C], bf16)
                for m in range(4):
                    wo32 = wst3.tile([P, C], f32, tag="wo32")
                    nc.sync.dma_start(wo32[:], wo_r[:, m, :])
                    nc.vector.tensor_copy(wor[:, m, :], wo32[:])

                for tb in range(NJB):
                    for cc in range(2):
                        psy = psB.tile([P, 512], f32, tag="pb")
                        for m in range(4):
                            nc.tensor.matmul(
                                psy[:],
                                oT[:, m, bass.ts(tb, P)],
                                wor[:, m, bass.ts(cc, 512)],
                                start=(m == 0),
                                stop=(m == 3),
                            )
                        ysb = ypool.tile([P, 512], f32, tag="ysb")
                        nc.vector.tensor_copy(ysb[:], psy[:])
                        nc.sync.dma_start(y_r[:, tb, bass.ts(cc, 512)], ysb[:])

    nc.compile()
    return nc


def kernel(x, Wq, Wk, Wv, Wo):
    global LAST_RESULTS, _NC_CACHE
    x = np.asarray(x, dtype=np.float32)
    Wq = np.asarray(Wq, dtype=np.float32)
    Wk = np.asarray(Wk, dtype=np.float32)
    Wv = np.asarray(Wv, dtype=np.float32)
    Wo = np.asarray(Wo, dtype=np.float32)

    slopes = np.asarray(get_slopes(NH), dtype=np.float32)
    ii = np.arange(T, dtype=np.float64)
    pp = np.arange(P, dtype=np.float64)

    if _NC_CACHE is None:
        _NC_CACHE = build_kernel()
    nc = _NC_CACHE

    in_maps = []
    for core in range(8):
        b, g = core // 2, core % 2
        hsl = slice(g * 512, (g + 1) * 512)
        core_slopes = slopes[g * NHC : (g + 1) * NHC].astype(np.float64)
        import ml_dtypes

        qaug1 = (-core_slopes[:, None] * ii[None, :]).astype(ml_dtypes.bfloat16)
        qaugb = np.ascontiguousarray(
            np.broadcast_to(qaug1[:, None, :], (8, NHC, T))
        )
        kaugb = np.zeros((8, NHC, T), ml_dtypes.bfloat16)
        for h in range(NHC):
            kaugb[h, h, :] = ml_dtypes.bfloat16(1.0)
        biasj = np.zeros((P, NHC, NJB), np.float32)
        for h in range(NHC):
            for jb in range(NJB):
                biasj[:, h, jb] = (core_slopes[h] * (128 * jb + pp)).astype(np.float32)
        in_maps.append(
            {
                "xT": np.ascontiguousarray(x[b].T),
                "wq": np.ascontiguousarray(Wq[:, hsl]) * np.float32(0.125),
                "wk": np.ascontiguousarray(Wk[:, hsl]),
                "wv": np.ascontiguousarray(Wv[:, hsl]),
                "wo": np.ascontiguousarray(Wo[hsl, :]),
                "qaugb": qaugb,
                "kaugb": kaugb,
                "biasj": biasj,
            }
        )

    res = run_bass_kernel_spmd(nc, in_maps, list(range(8)))
    LAST_RESULTS = res
    out = np.empty((B, T, C), dtype=np.float32)
    for b in range(B):
        out[b] = res.results[2 * b]["y"] + res.results[2 * b + 1]["y"]
    return out



# revision 11
# speedup vs baseline: 1.7453x; 1.7453x over previous
"""Causal attention with ALiBi for nn_CausalAttention (B=4, T=2048, C=1024,
16 heads) on 8 TRN2 NeuronCores.

Sharding: batch (4) x head-group (2 groups of 8 heads) -> 8 cores, with
heads interleaved even/odd across the two groups so that head-slot s holds
original heads (2s, 2s+1) on groups (0, 1). ALiBi slopes decay
geometrically with head index, so slot s only needs keys within a window
W_s = 16 * 2^(s+1) positions back (contributions beyond are < e^-16
relative); score tiles outside the window are skipped entirely.

Per core (one batch b, one head group g), phased for continuous PE
streaming:
  A: load x -> bf16 SBUF (once), weights -> bf16 SBUF.
  B: v projection -> vaug [j, slot, hd+ones]; q/k projections in [d, t]
     layout via PSUM->bf16 cast + SBUF-to-SBUF DMA into qT2/kT2 (rows
     64-71 hold augmented ALiBi rows: kaug one-hot per slot, qaug
     -slope*i which cancels per-row in softmax; K=72 contracts both).
  C: per slot, per key-block jb: QK matmuls over the windowed i-chunks
     (diagonal chunk narrowed to skip fully-masked columns), exp via ACT
     with bias +slope*j (exact fp32), causal mask by gpsimd affine_select
     on the [128,128] diagonal strip only (also kills Inf); after each
     4th jb, PV for the completed i-chunk with an appended ones column
     for the softmax denominator; normalize via vector.reciprocal +
     gpsimd partition_broadcast into oT.
  D: y = oT.T @ Wo_rows per t-block.
Host sums the two head-group partials per batch.
"""

import math

import numpy as np

import concourse.bass as bass
import concourse.mybir as mybir
import concourse.tile as tile
from concourse import bacc
from concourse.bass_utils import run_bass_kernel_spmd

B, T, C = 4, 2048, 1024
NH, HD = 16, 64
NHC = 8  # head-slots per core
NJB = T // 128  # 16 key blocks
NCH = T // 512  # 4 query chunks
P = 128

f32 = mybir.dt.float32
bf16 = mybir.dt.bfloat16

# per-slot attention window (keys further back contribute < e^-16 rel):
# slot s holds original heads (2s, 2s+1); binding slope = 2^-(s+1).
WIN = [32, 64, 128, 256, 512, 1024, 2048, 4096]

# last i-chunk covered by (slot, jb): include chunk c iff its first query
# can see block jb: 512c <= 128jb + 127 + W.
C1 = [
    [min(NCH - 1, (128 * jb + 127 + WIN[s]) // 512) for jb in range(NJB)]
    for s in range(NHC)
]
# first key block contributing to chunk c (same inequality, inverted)
JBMIN = [[0] * NCH for _ in range(NHC)]
for _s in range(NHC):
    for _c in range(NCH):
        _jm = 0
        while C1[_s][_jm] < _c:
            _jm += 1
        JBMIN[_s][_c] = _jm

LAST_RESULTS = None
_NC_CACHE = None


def get_slopes(n):
    def pow2(n):
        start = 2 ** (-(2 ** (-(math.log2(n) - 3))))
        return [start * start**i for i in range(n)]

    if math.log2(n).is_integer():
        return pow2(n)
    c = 2 ** math.floor(math.log2(n))
    return pow2(c) + get_slopes(2 * c)[0::2][: n - c]


def _select_diag(nc, ap):
    """causal mask on a [128, 128] diagonal strip: keep col - part >= 0."""
    nc.gpsimd.affine_select(
        ap,
        ap,
        pattern=[[1, 128]],
        compare_op=mybir.AluOpType.is_ge,
        fill=0.0,
        base=0,
        channel_multiplier=-1,
    )


def _emit_norm(nc, npool, oT, s, pot, c):
    """softmax denominator: row 64 of pot; normalize rows 0..63 into oT."""
    hp = (s % 2) * 64
    hm = s // 2
    potsb = npool.tile([65, 512], f32, tag="potsb")
    nc.vector.tensor_copy(potsb[:], pot[:])
    # spread the 512 rowsums across 128 partitions for a fast reciprocal
    rs128 = npool.tile([P, 4], f32, tag="rs")
    nc.sync.dma_start(rs128[:], potsb[64:65, :])
    nc.vector.reciprocal(rs128[:], rs128[:])
    srecip = npool.tile([1, 512], f32, tag="sr")
    nc.sync.dma_start(srecip[:], rs128[:])
    bcast = npool.tile([64, 512], f32, tag="bc")
    nc.gpsimd.partition_broadcast(bcast[:], srecip[:])
    nc.vector.tensor_tensor(
        oT[hp : hp + 64, hm, bass.ts(c, 512)],
        potsb[0:64, :],
        bcast[:],
        mybir.AluOpType.mult,
    )


def build_kernel():
    nc = bacc.Bacc("TRN2", target_bir_lowering=False, debug=False, num_devices=8)

    xT_d = nc.dram_tensor("xT", [C, T], f32, kind="ExternalInput").ap()
    wq_d = nc.dram_tensor("wq", [C, 512], f32, kind="ExternalInput").ap()
    wk_d = nc.dram_tensor("wk", [C, 512], f32, kind="ExternalInput").ap()
    wv_d = nc.dram_tensor("wv", [C, 512], f32, kind="ExternalInput").ap()
    wo_d = nc.dram_tensor("wo", [512, C], f32, kind="ExternalInput").ap()
    qaug_d = nc.dram_tensor("qaugb", [8, NHC, T], bf16, kind="ExternalInput").ap()
    kaug_d = nc.dram_tensor("kaugb", [8, NHC, T], bf16, kind="ExternalInput").ap()
    biasj_d = nc.dram_tensor("biasj", [P, NHC, NJB], f32, kind="ExternalInput").ap()
    y_d = nc.dram_tensor("y", [T, C], f32, kind="ExternalOutput").ap()

    xT_r = xT_d.rearrange("(cb p) t -> p cb t", p=P)  # [128, 8, 2048]
    wq_r = wq_d.rearrange("(cb p) m -> p cb m", p=P)  # [128, 8, 512]
    wk_r = wk_d.rearrange("(cb p) m -> p cb m", p=P)
    wv_r = wv_d.rearrange("(cb p) m -> p cb m", p=P)
    # [128, 4, 2, 512]: (pair m, 512-col half cc)
    wo_r = wo_d.rearrange("(mb p) (a n) -> p mb a n", p=P, a=2)
    y_r = y_d.rearrange("(tb p) c -> p tb c", p=P)  # [128, 16, 1024]

    with tile.TileContext(nc) as tc:
        with tc.tile_pool(name="persist", bufs=1) as persist:
            qT2 = persist.tile([72, NHC, T], bf16)
            kT2 = persist.tile([72, NHC, T], bf16)
            vaug = persist.tile([P, NJB, NHC, 66], bf16)
            oT = persist.tile([P, 4, T], bf16)
            biasj = persist.tile([P, NHC, NJB], f32)
            wob = persist.tile([P, 4, 2, 512], bf16)

            nc.gpsimd.memset(vaug[:, :, :, 64:66], 1.0)
            nc.sync.dma_start(biasj[:], biasj_d[:])
            nc.sync.dma_start(kT2[64:72, :, :], kaug_d[:])
            nc.sync.dma_start(qT2[64:72, :, :], qaug_d[:])

            # ---- phase A+B: load + projections ----
            with (
                tc.tile_pool(name="stage", bufs=1) as stage,
                tc.tile_pool(name="qkst", bufs=2) as qkst,
                tc.tile_pool(name="st32", bufs=2) as st32,
                tc.tile_pool(name="psV", bufs=2, space="PSUM") as psV,
                tc.tile_pool(name="psQ", bufs=2, space="PSUM") as psQ,
                tc.tile_pool(name="psK", bufs=2, space="PSUM") as psK,
            ):
                xb = stage.tile([P, 8, T], bf16, tag="xb")
                wvb = stage.tile([P, 8, 512], bf16, tag="wvb")
                wqb = stage.tile([P, 8, 512], bf16, tag="wqb")
                wkb = stage.tile([P, 8, 512], bf16, tag="wkb")

                # wv then x (v projection starts as soon as tck 0 lands)
                for half in range(2):
                    w32 = st32.tile([P, 4, 512], f32, tag="w32")
                    nc.sync.dma_start(w32[:], wv_r[:, 4 * half : 4 * half + 4, :])
                    nc.vector.tensor_copy(
                        wvb[:, 4 * half : 4 * half + 4, :], w32[:]
                    )
                for tck in range(NCH):
                    for cb in range(8):
                        x32 = st32.tile([P, 512], f32, tag="x32")
                        nc.sync.dma_start(x32[:], xT_r[:, cb, bass.ts(tck, 512)])
                        nc.vector.tensor_copy(xb[:, cb, bass.ts(tck, 512)], x32[:])
                for wsrc, wdst in ((wq_r, wqb), (wk_r, wkb)):
                    for half in range(2):
                        w32 = st32.tile([P, 4, 512], f32, tag="w32")
                        nc.sync.dma_start(w32[:], wsrc[:, 4 * half : 4 * half + 4, :])
                        nc.vector.tensor_copy(
                            wdst[:, 4 * half : 4 * half + 4, :], w32[:]
                        )
                for half in range(2):
                    wo32 = st32.tile([P, 4, 512], f32, tag="w32")
                    nc.sync.dma_start(
                        wo32[:].rearrange("p (a b) n -> p a b n", a=2),
                        wo_r[:, 2 * half : 2 * half + 2, :, :],
                    )
                    nc.vector.tensor_copy(
                        wob[:, 2 * half : 2 * half + 2, :, :],
                        wo32[:].rearrange("p (a b) n -> p a b n", a=2),
                    )

                # v projection: psv[t, slot*hd] per 128-t block
                for tck in range(NCH):
                    for tb in range(4):
                        psv = psV.tile([P, 512], f32, tag="pv")
                        for c in range(8):
                            nc.tensor.matmul(
                                psv[:],
                                xb[:, c, 512 * tck + 128 * tb : 512 * tck + 128 * (tb + 1)],
                                wvb[:, c, :],
                                start=(c == 0),
                                stop=(c == 7),
                            )
                        nc.vector.tensor_copy(
                            vaug[:, 4 * tck + tb, :, 0:64],
                            psv[:].rearrange("p (h d) -> p h d", h=NHC),
                        )

                # q/k projections per slot pair, [d, t] layout via DMA
                for m in range(4):
                    qsg = qkst.tile([P, T], bf16, tag="qsg")
                    ksg = qkst.tile([P, T], bf16, tag="ksg")
                    for tck in range(NCH):
                        psq = psQ.tile([P, 512], f32, tag="pq")
                        psk = psK.tile([P, 512], f32, tag="pk")
                        for c in range(8):
                            nc.tensor.matmul(
                                psq[:],
                                wqb[:, c, bass.ts(m, P)],
                                xb[:, c, bass.ts(tck, 512)],
                                start=(c == 0),
                                stop=(c == 7),
                            )
                            nc.tensor.matmul(
                                psk[:],
                                wkb[:, c, bass.ts(m, P)],
                                xb[:, c, bass.ts(tck, 512)],
                                start=(c == 0),
                                stop=(c == 7),
                            )
                        nc.vector.tensor_copy(qsg[:, bass.ts(tck, 512)], psq[:])
                        nc.vector.tensor_copy(ksg[:, bass.ts(tck, 512)], psk[:])
                    nc.sync.dma_start(qT2[0:64, 2 * m, :], qsg[0:64, :])
                    nc.sync.dma_start(qT2[0:64, 2 * m + 1, :], qsg[64:128, :])
                    nc.sync.dma_start(kT2[0:64, 2 * m, :], ksg[0:64, :])
                    nc.sync.dma_start(kT2[0:64, 2 * m + 1, :], ksg[64:128, :])

            # ---- phase C: attention ----
            with (
                tc.tile_pool(name="ptd_p", bufs=16) as ptd_p,
                tc.tile_pool(name="pt1_p", bufs=16) as pt1_p,
                tc.tile_pool(name="pt2_p", bufs=14) as pt2_p,
                tc.tile_pool(name="npool", bufs=3) as npool,
                tc.tile_pool(name="psD", bufs=2, space="PSUM") as psD,
                tc.tile_pool(name="psA2", bufs=2, space="PSUM") as psA2,
                tc.tile_pool(name="psC", bufs=2, space="PSUM") as psC,
            ):
                for s in range(NHC):
                    # (jb, c) -> (tile, mid_idx or None, width, pot col offset)
                    pt_reg = {}
                    deferred_norm = []

                    for jb in range(NJB):
                        c0 = jb // 4
                        r = jb % 4
                        c1 = C1[s][jb]
                        if r > 0:
                            # diagonal chunk, narrowed: cols 128r..512 of c0
                            w = 512 - 128 * r
                            sd = psD.tile([P, 512], f32, tag="sd")
                            nc.tensor.matmul(
                                sd[:, 0:w],
                                kT2[:, s, bass.ts(jb, P)],
                                qT2[:, s, 512 * c0 + 128 * r : 512 * (c0 + 1)],
                                start=True,
                                stop=True,
                            )
                            td = ptd_p.tile([P, 512], bf16, tag="ptd")
                            nc.scalar.activation(
                                td[:, 0:w],
                                sd[:, 0:w],
                                mybir.ActivationFunctionType.Exp,
                                bias=biasj[:, s, jb : jb + 1],
                                scale=1.0,
                            )
                            _select_diag(nc, td[:, 0:128])
                            pt_reg[(jb, c0)] = (td, None, w, 128 * r)
                            fulls = list(range(c0 + 1, c1 + 1))
                        else:
                            fulls = list(range(c0, c1 + 1))

                        g = 0
                        while g < len(fulls):
                            ng = min(2, len(fulls) - g)
                            if ng == 2:
                                s2 = psA2.tile([P, 2, 512], f32, tag="sa")
                                t2 = pt2_p.tile([P, 2, 512], bf16, tag="pt2")
                                for i in range(2):
                                    nc.tensor.matmul(
                                        s2[:, i, :],
                                        kT2[:, s, bass.ts(jb, P)],
                                        qT2[:, s, bass.ts(fulls[g + i], 512)],
                                        start=True,
                                        stop=True,
                                    )
                                nc.scalar.activation(
                                    t2[:],
                                    s2[:],
                                    mybir.ActivationFunctionType.Exp,
                                    bias=biasj[:, s, jb : jb + 1],
                                    scale=1.0,
                                )
                                for i in range(2):
                                    pt_reg[(jb, fulls[g + i])] = (t2, i, 512, 0)
                            else:
                                s1 = psD.tile([P, 512], f32, tag="sd")
                                t1 = pt1_p.tile([P, 512], bf16, tag="pt1")
                                nc.tensor.matmul(
                                    s1[:],
                                    kT2[:, s, bass.ts(jb, P)],
                                    qT2[:, s, bass.ts(fulls[g], 512)],
                                    start=True,
                                    stop=True,
                                )
                                nc.scalar.activation(
                                    t1[:],
                                    s1[:],
                                    mybir.ActivationFunctionType.Exp,
                                    bias=biasj[:, s, jb : jb + 1],
                                    scale=1.0,
                                )
                                pt_reg[(jb, fulls[g])] = (t1, None, 512, 0)
                            g += ng

                        if r == 0:
                            tl, mi, _, _ = pt_reg[(jb, c0)]
                            sel_ap = tl[:, 0:128] if mi is None else tl[:, mi, 0:128]
                            _select_diag(nc, sel_ap)

                        # emit the previous chunk's normalization here so the
                        # gpsimd broadcast sits behind this jb's select, not
                        # ahead of it (keeps the gpsimd queue from stalling
                        # the next select on the reciprocal DMA chain).
                        if deferred_norm:
                            _emit_norm(nc, npool, oT, s, *deferred_norm.pop())

                        if (jb + 1) % 4 == 0:
                            c = jb // 4
                            jmin = JBMIN[s][c]
                            njb = 4 * c + 4 - jmin
                            pot = psC.tile([65, 512], f32, tag="pot")
                            for idx, jbp in enumerate(range(jmin, 4 * c + 4)):
                                tl, mi, w, off = pt_reg.pop((jbp, c))
                                mov = tl[:, 0:w] if mi is None else tl[:, mi, 0:w]
                                nc.tensor.matmul(
                                    pot[:, off : off + w],
                                    vaug[:, jbp, s, 0:65],
                                    mov,
                                    start=(idx == 0),
                                    stop=(idx == njb - 1),
                                )
                            deferred_norm.append((pot, c))

                    # last chunk's normalization
                    if deferred_norm:
                        _emit_norm(nc, npool, oT, s, *deferred_norm.pop())

            # ---- phase D: output projection ----
            with (
                tc.tile_pool(name="ypool", bufs=4) as ypool,
                tc.tile_pool(name="psY", bufs=4, space="PSUM") as psY,
            ):
                for tb in range(NJB):
                    for cc in range(2):
                        psy = psY.tile([P, 512], f32, tag="py")
                        for m in range(4):
                            nc.tensor.matmul(
                                psy[:],
                                oT[:, m, bass.ts(tb, P)],
                                wob[:, m, cc, :],
                                start=(m == 0),
                                stop=(m == 3),
                            )
                        ysb = ypool.tile([P, 512], f32, tag="ysb")
                        nc.vector.tensor_copy(ysb[:], psy[:])
                        nc.sync.dma_start(y_r[:, tb, bass.ts(cc, 512)], ysb[:])

    nc.compile()
    return nc


def kernel(x, Wq, Wk, Wv, Wo):
    global LAST_RESULTS, _NC_CACHE
    import ml_dtypes

    x = np.asarray(x, dtype=np.float32)
    Wq = np.asarray(Wq, dtype=np.float32)
    Wk = np.asarray(Wk, dtype=np.float32)
    Wv = np.asarray(Wv, dtype=np.float32)
    Wo = np.asarray(Wo, dtype=np.float32)

    slopes = np.asarray(get_slopes(NH), dtype=np.float64)
    ii = np.arange(T, dtype=np.float64)
    pp = np.arange(P, dtype=np.float64)

    if _NC_CACHE is None:
        _NC_CACHE = build_kernel()
    nc = _NC_CACHE

    in_maps = []
    for core in range(8):
        b, g = core // 2, core % 2
        perm = list(range(g, NH, 2))  # slot s -> original head 2s+g
        core_slopes = slopes[perm]

        qaug1 = (-core_slopes[:, None] * ii[None, :]).astype(ml_dtypes.bfloat16)
        qaugb = np.ascontiguousarray(np.broadcast_to(qaug1[:, None, :], (8, NHC, T)))
        kaugb = np.zeros((8, NHC, T), ml_dtypes.bfloat16)
        for h in range(NHC):
            kaugb[h, h, :] = ml_dtypes.bfloat16(1.0)
        biasj = np.zeros((P, NHC, NJB), np.float32)
        for h in range(NHC):
            for jb in range(NJB):
                biasj[:, h, jb] = (core_slopes[h] * (128 * jb + pp)).astype(np.float32)

        wq_g = np.concatenate([Wq[:, 64 * h : 64 * h + 64] for h in perm], axis=1)
        wk_g = np.concatenate([Wk[:, 64 * h : 64 * h + 64] for h in perm], axis=1)
        wv_g = np.concatenate([Wv[:, 64 * h : 64 * h + 64] for h in perm], axis=1)
        wo_g = np.concatenate([Wo[64 * h : 64 * h + 64, :] for h in perm], axis=0)

        in_maps.append(
            {
                "xT": np.ascontiguousarray(x[b].T),
                "wq": np.ascontiguousarray(wq_g) * np.float32(0.125),
                "wk": np.ascontiguousarray(wk_g),
                "wv": np.ascontiguousarray(wv_g),
                "wo": np.ascontiguousarray(wo_g),
                "qaugb": qaugb,
                "kaugb": kaugb,
                "biasj": biasj,
            }
        )

    res = run_bass_kernel_spmd(nc, in_maps, list(range(8)))
    LAST_RESULTS = res
    out = np.empty((B, T, C), dtype=np.float32)
    for b in range(B):
        out[b] = res.results[2 * b]["y"] + res.results[2 * b + 1]["y"]
    return out


# revision 16
# speedup vs baseline: 1.9651x; 1.1259x over previous
"""Causal attention with ALiBi for nn_CausalAttention (B=4, T=2048, C=1024,
16 heads) on 8 TRN2 NeuronCores.

Sharding: batch (4) x head-group (2 groups of 8 heads) -> 8 cores, with
heads interleaved even/odd across the two groups so that head-slot s holds
original heads (2s, 2s+1) on groups (0, 1). ALiBi slopes decay
geometrically with head index, so slot s only needs keys within a window
W_s = 16 * 2^(s+1) positions back (contributions beyond are < e^-16
relative); score tiles outside the window are skipped entirely.

Per core (one batch b, one head group g), phased for continuous PE
streaming:
  A: load x -> bf16 SBUF (once), weights -> bf16 SBUF.
  B: v projection -> vaug [j, slot, hd+ones]; q/k projections in [d, t]
     layout via PSUM->bf16 cast + SBUF-to-SBUF DMA into qT2/kT2 (rows
     64-71 hold augmented ALiBi rows: kaug one-hot per slot, qaug
     -slope*i which cancels per-row in softmax; K=72 contracts both).
  C: per slot, per key-block jb: QK matmuls over the windowed i-chunks
     (diagonal chunk narrowed to skip fully-masked columns), exp via ACT
     with bias +slope*j (exact fp32), causal mask by gpsimd affine_select
     on the [128,128] diagonal strip only (also kills Inf); after each
     4th jb, PV for the completed i-chunk with an appended ones column
     for the softmax denominator; normalize via vector.reciprocal +
     gpsimd partition_broadcast into oT.
  D: y = oT.T @ Wo_rows per t-block.
Host sums the two head-group partials per batch.
"""

import math

import numpy as np

import concourse.bass as bass
import concourse.mybir as mybir
import concourse.tile as tile
from concourse import bacc
from concourse.bass_utils import run_bass_kernel_spmd

B, T, C = 4, 2048, 1024
NH, HD = 16, 64
NHC = 8  # head-slots per core
NJB = T // 128  # 16 key blocks
NCH = T // 512  # 4 query chunks
P = 128

f32 = mybir.dt.float32
bf16 = mybir.dt.bfloat16

# per-slot attention window (keys further back contribute < e^-16 rel):
# slot s holds original heads (2s, 2s+1); binding slope = 2^-(s+1).
WIN = [32, 64, 128, 256, 512, 1024, 2048, 4096]

# last i-chunk covered by (slot, jb): include chunk c iff its first query
# can see block jb: 512c <= 128jb + 127 + W.
C1 = [
    [min(NCH - 1, (128 * jb + 127 + WIN[s]) // 512) for jb in range(NJB)]
    for s in range(NHC)
]
# first key block contributing to chunk c (same inequality, inverted)
JBMIN = [[0] * NCH for _ in range(NHC)]
for _s in range(NHC):
    for _c in range(NCH):
        _jm = 0
        while C1[_s][_jm] < _c:
            _jm += 1
        JBMIN[_s][_c] = _jm

LAST_RESULTS = None
_NC_CACHE = None


def get_slopes(n):
    def pow2(n):
        start = 2 ** (-(2 ** (-(math.log2(n) - 3))))
        return [start * start**i for i in range(n)]

    if math.log2(n).is_integer():
        return pow2(n)
    c = 2 ** math.floor(math.log2(n))
    return pow2(c) + get_slopes(2 * c)[0::2][: n - c]


def _select_diag(nc, ap):
    """causal mask on a [128, 128] diagonal strip: keep col - part >= 0."""
    nc.gpsimd.affine_select(
        ap,
        ap,
        pattern=[[1, 128]],
        compare_op=mybir.AluOpType.is_ge,
        fill=0.0,
        base=0,
        channel_multiplier=-1,
    )


def _emit_norm(nc, npool, oT, s, pot, c):
    """softmax denominator: row 64 of pot; normalize rows 0..63 into oT."""
    hp = (s % 2) * 64
    hm = s // 2
    potsb = npool.tile([65, 512], f32, tag="potsb")
    nc.vector.tensor_copy(potsb[:], pot[:])
    # spread the 512 rowsums across 128 partitions for a fast reciprocal
    rs128 = npool.tile([P, 4], f32, tag="rs")
    nc.sync.dma_start(rs128[:], potsb[64:65, :])
    nc.vector.reciprocal(rs128[:], rs128[:])
    srecip = npool.tile([1, 512], f32, tag="sr")
    nc.sync.dma_start(srecip[:], rs128[:])
    bcast = npool.tile([64, 512], f32, tag="bc")
    nc.gpsimd.partition_broadcast(bcast[:], srecip[:])
    nc.vector.tensor_tensor(
        oT[hp : hp + 64, hm, bass.ts(c, 512)],
        potsb[0:64, :],
        bcast[:],
        mybir.AluOpType.mult,
    )


def build_kernel():
    nc = bacc.Bacc("TRN2", target_bir_lowering=False, debug=False, num_devices=8)

    xT_d = nc.dram_tensor("xT", [C, T], f32, kind="ExternalInput").ap()
    wq_d = nc.dram_tensor("wq", [C, 512], f32, kind="ExternalInput").ap()
    wk_d = nc.dram_tensor("wk", [C, 512], f32, kind="ExternalInput").ap()
    wv_d = nc.dram_tensor("wv", [C, 512], f32, kind="ExternalInput").ap()
    wo_d = nc.dram_tensor("wo", [512, C], f32, kind="ExternalInput").ap()
    qaug_d = nc.dram_tensor("qaugb", [8, NHC, T], bf16, kind="ExternalInput").ap()
    kaug_d = nc.dram_tensor("kaugb", [8, NHC, T], bf16, kind="ExternalInput").ap()
    biasj_d = nc.dram_tensor("biasj", [P, NHC, NJB], f32, kind="ExternalInput").ap()
    y_d = nc.dram_tensor("y", [T, C], f32, kind="ExternalOutput").ap()

    xT_r = xT_d.rearrange("(cb p) t -> p cb t", p=P)  # [128, 8, 2048]
    wq_r = wq_d.rearrange("(cb p) m -> p cb m", p=P)  # [128, 8, 512]
    wk_r = wk_d.rearrange("(cb p) m -> p cb m", p=P)
    wv_r = wv_d.rearrange("(cb p) m -> p cb m", p=P)
    # [128, 4, 2, 512]: (pair m, 512-col half cc)
    wo_r = wo_d.rearrange("(mb p) (a n) -> p mb a n", p=P, a=2)
    y_r = y_d.rearrange("(tb p) c -> p tb c", p=P)  # [128, 16, 1024]

    with tile.TileContext(nc) as tc:
        with tc.tile_pool(name="persist", bufs=1) as persist:
            qT2 = persist.tile([72, NHC, T], bf16)
            kT2 = persist.tile([72, NHC, T], bf16)
            vaug = persist.tile([P, NJB, NHC, 66], bf16)
            oT = persist.tile([P, 4, T], bf16)
            biasj = persist.tile([P, NHC, NJB], f32)
            wob = persist.tile([P, 4, 2, 512], bf16)

            nc.gpsimd.memset(vaug[:, :, :, 64:66], 1.0)
            nc.sync.dma_start(biasj[:], biasj_d[:])
            nc.sync.dma_start(kT2[64:72, :, :], kaug_d[:])
            nc.sync.dma_start(qT2[64:72, :, :], qaug_d[:])

            # ---- phase A+B: load + projections ----
            with (
                tc.tile_pool(name="stage", bufs=1) as stage,
                tc.tile_pool(name="qkst", bufs=2) as qkst,
                tc.tile_pool(name="st32", bufs=2) as st32,
                tc.tile_pool(name="psV", bufs=2, space="PSUM") as psV,
                tc.tile_pool(name="psQ", bufs=2, space="PSUM") as psQ,
                tc.tile_pool(name="psK", bufs=2, space="PSUM") as psK,
            ):
                xb = stage.tile([P, 8, T], bf16, tag="xb")
                wvb = stage.tile([P, 8, 512], bf16, tag="wvb")
                wqb = stage.tile([P, 8, 512], bf16, tag="wqb")
                wkb = stage.tile([P, 8, 512], bf16, tag="wkb")

                # wv then x (v projection starts as soon as tck 0 lands)
                for half in range(2):
                    w32 = st32.tile([P, 4, 512], f32, tag="w32")
                    nc.sync.dma_start(w32[:], wv_r[:, 4 * half : 4 * half + 4, :])
                    nc.vector.tensor_copy(
                        wvb[:, 4 * half : 4 * half + 4, :], w32[:]
                    )
                for tck in range(NCH):
                    for cb in range(8):
                        x32 = st32.tile([P, 512], f32, tag="x32", bufs=6)
                        nc.sync.dma_start(x32[:], xT_r[:, cb, bass.ts(tck, 512)])
                        nc.vector.tensor_copy(xb[:, cb, bass.ts(tck, 512)], x32[:])
                for wsrc, wdst in ((wq_r, wqb), (wk_r, wkb)):
                    for half in range(2):
                        w32 = st32.tile([P, 4, 512], f32, tag="w32")
                        nc.sync.dma_start(w32[:], wsrc[:, 4 * half : 4 * half + 4, :])
                        nc.vector.tensor_copy(
                            wdst[:, 4 * half : 4 * half + 4, :], w32[:]
                        )
                for half in range(2):
                    wo32 = st32.tile([P, 4, 512], f32, tag="w32")
                    nc.sync.dma_start(
                        wo32[:].rearrange("p (a b) n -> p a b n", a=2),
                        wo_r[:, 2 * half : 2 * half + 2, :, :],
                    )
                    nc.vector.tensor_copy(
                        wob[:, 2 * half : 2 * half + 2, :, :],
                        wo32[:].rearrange("p (a b) n -> p a b n", a=2),
                    )

                # v projection: psv[t, slot*hd] per 128-t block
                for tck in range(NCH):
                    for tb in range(4):
                        psv = psV.tile([P, 512], f32, tag="pv")
                        for c in range(8):
                            nc.tensor.matmul(
                                psv[:],
                                xb[:, c, 512 * tck + 128 * tb : 512 * tck + 128 * (tb + 1)],
                                wvb[:, c, :],
                                start=(c == 0),
                                stop=(c == 7),
                            )
                        nc.vector.tensor_copy(
                            vaug[:, 4 * tck + tb, :, 0:64],
                            psv[:].rearrange("p (h d) -> p h d", h=NHC),
                        )

                # q/k projections per slot pair, [d, t] layout via DMA
                for m in range(4):
                    qsg = qkst.tile([P, T], bf16, tag="qsg")
                    ksg = qkst.tile([P, T], bf16, tag="ksg")
                    for tck in range(NCH):
                        psq = psQ.tile([P, 512], f32, tag="pq")
                        psk = psK.tile([P, 512], f32, tag="pk")
                        for c in range(8):
                            nc.tensor.matmul(
                                psq[:],
                                wqb[:, c, bass.ts(m, P)],
                                xb[:, c, bass.ts(tck, 512)],
                                start=(c == 0),
                                stop=(c == 7),
                            )
                            nc.tensor.matmul(
                                psk[:],
                                wkb[:, c, bass.ts(m, P)],
                                xb[:, c, bass.ts(tck, 512)],
                                start=(c == 0),
                                stop=(c == 7),
                            )
                        nc.vector.tensor_copy(qsg[:, bass.ts(tck, 512)], psq[:])
                        nc.vector.tensor_copy(ksg[:, bass.ts(tck, 512)], psk[:])
                    nc.sync.dma_start(qT2[0:64, 2 * m, :], qsg[0:64, :])
                    nc.sync.dma_start(qT2[0:64, 2 * m + 1, :], qsg[64:128, :])
                    nc.sync.dma_start(kT2[0:64, 2 * m, :], ksg[0:64, :])
                    nc.sync.dma_start(kT2[0:64, 2 * m + 1, :], ksg[64:128, :])

            # ---- phase C: attention ----
            with (
                tc.tile_pool(name="ptd_p", bufs=16) as ptd_p,
                tc.tile_pool(name="pt1_p", bufs=16) as pt1_p,
                tc.tile_pool(name="pt2_p", bufs=14) as pt2_p,
                tc.tile_pool(name="npool", bufs=3) as npool,
                tc.tile_pool(name="psD", bufs=3, space="PSUM") as psD,
                tc.tile_pool(name="psA2", bufs=2, space="PSUM") as psA2,
                tc.tile_pool(name="psC", bufs=1, space="PSUM") as psC,
            ):
                # (s, jb, c) -> (tile, mid_idx or None, width, pot col offset)
                pt_reg = {}
                pend_pv = []  # chunks whose PV is deferred one jb step
                pend_norm = []  # pots whose normalization is deferred one more

                def emit_pv(s, c):
                    jmin = JBMIN[s][c]
                    njb = 4 * c + 4 - jmin
                    pot = psC.tile([65, 512], f32, tag="pot")
                    for idx, jbp in enumerate(range(jmin, 4 * c + 4)):
                        tl, mi, w, off = pt_reg.pop((s, jbp, c))
                        mov = tl[:, 0:w] if mi is None else tl[:, mi, 0:w]
                        nc.tensor.matmul(
                            pot[:, off : off + w],
                            vaug[:, jbp, s, 0:65],
                            mov,
                            start=(idx == 0),
                            stop=(idx == njb - 1),
                        )
                    pend_norm.append((s, pot, c))

                for s in range(NHC):
                    for jb in range(NJB):
                        c0 = jb // 4
                        r = jb % 4
                        c1 = C1[s][jb]
                        if r > 0:
                            # diagonal chunk, narrowed: cols 128r..512 of c0
                            w = 512 - 128 * r
                            sd = psD.tile([P, 512], f32, tag="sd")
                            nc.tensor.matmul(
                                sd[:, 0:w],
                                kT2[:, s, bass.ts(jb, P)],
                                qT2[:, s, 512 * c0 + 128 * r : 512 * (c0 + 1)],
                                start=True,
                                stop=True,
                            )
                            td = ptd_p.tile([P, 512], bf16, tag="ptd")
                            nc.scalar.activation(
                                td[:, 0:w],
                                sd[:, 0:w],
                                mybir.ActivationFunctionType.Exp,
                                bias=biasj[:, s, jb : jb + 1],
                                scale=1.0,
                            )
                            _select_diag(nc, td[:, 0:128])
                            pt_reg[(s, jb, c0)] = (td, None, w, 128 * r)
                            fulls = list(range(c0 + 1, c1 + 1))
                        else:
                            fulls = list(range(c0, c1 + 1))

                        g = 0
                        while g < len(fulls):
                            ng = min(2, len(fulls) - g)
                            if ng == 2:
                                s2 = psA2.tile([P, 2, 512], f32, tag="sa")
                                t2 = pt2_p.tile([P, 2, 512], bf16, tag="pt2")
                                for i in range(2):
                                    nc.tensor.matmul(
                                        s2[:, i, :],
                                        kT2[:, s, bass.ts(jb, P)],
                                        qT2[:, s, bass.ts(fulls[g + i], 512)],
                                        start=True,
                                        stop=True,
                                    )
                                nc.scalar.activation(
                                    t2[:],
                                    s2[:],
                                    mybir.ActivationFunctionType.Exp,
                                    bias=biasj[:, s, jb : jb + 1],
                                    scale=1.0,
                                )
                                for i in range(2):
                                    pt_reg[(s, jb, fulls[g + i])] = (t2, i, 512, 0)
                            else:
                                s1 = psD.tile([P, 512], f32, tag="sd")
                                t1 = pt1_p.tile([P, 512], bf16, tag="pt1")
                                nc.tensor.matmul(
                                    s1[:],
                                    kT2[:, s, bass.ts(jb, P)],
                                    qT2[:, s, bass.ts(fulls[g], 512)],
                                    start=True,
                                    stop=True,
                                )
                                nc.scalar.activation(
                                    t1[:],
                                    s1[:],
                                    mybir.ActivationFunctionType.Exp,
                                    bias=biasj[:, s, jb : jb + 1],
                                    scale=1.0,
                                )
                                pt_reg[(s, jb, fulls[g])] = (t1, None, 512, 0)
                            g += ng

                        if r == 0:
                            tl, mi, _, _ = pt_reg[(s, jb, c0)]
                            sel_ap = tl[:, 0:128] if mi is None else tl[:, mi, 0:128]
                            _select_diag(nc, sel_ap)

                        # flush work deferred from the previous jb step: the
                        # PV chain (hides the exp/select latency under this
                        # jb's QK) and then the normalization deferred from
                        # the step before (keeps the gpsimd broadcast behind
                        # this jb's select).
                        if pend_norm:
                            ns_, pot_, nc_ = pend_norm.pop(0)
                            _emit_norm(nc, npool, oT, ns_, pot_, nc_)
                        if pend_pv:
                            emit_pv(*pend_pv.pop(0))
                        if (jb + 1) % 4 == 0:
                            pend_pv.append((s, jb // 4))

                # drain the tail
                while pend_pv or pend_norm:
                    if pend_norm:
                        ns_, pot_, nc_ = pend_norm.pop(0)
                        _emit_norm(nc, npool, oT, ns_, pot_, nc_)
                    if pend_pv:
                        emit_pv(*pend_pv.pop(0))

            # ---- phase D: output projection ----
            with (
                tc.tile_pool(name="ypool", bufs=4) as ypool,
                tc.tile_pool(name="psY", bufs=4, space="PSUM") as psY,
            ):
                for tb in range(NJB):
                    for cc in range(2):
                        psy = psY.tile([P, 512], f32, tag="py")
                        for m in range(4):
                            nc.tensor.matmul(
                                psy[:],
                                oT[:, m, bass.ts(tb, P)],
                                wob[:, m, cc, :],
                                start=(m == 0),
                                stop=(m == 3),
                            )
                        ysb = ypool.tile([P, 512], f32, tag="ysb")
                        nc.vector.tensor_copy(ysb[:], psy[:])
                        nc.sync.dma_start(y_r[:, tb, bass.ts(cc, 512)], ysb[:])

    nc.compile()
    return nc


def kernel(x, Wq, Wk, Wv, Wo):
    global LAST_RESULTS, _NC_CACHE
    import ml_dtypes

    x = np.asarray(x, dtype=np.float32)
    Wq = np.asarray(Wq, dtype=np.float32)
    Wk = np.asarray(Wk, dtype=np.float32)
    Wv = np.asarray(Wv, dtype=np.float32)
    Wo = np.asarray(Wo, dtype=np.float32)

    slopes = np.asarray(get_slopes(NH), dtype=np.float64)
    ii = np.arange(T, dtype=np.float64)
    pp = np.arange(P, dtype=np.float64)

    if _NC_CACHE is None:
        _NC_CACHE = build_kernel()
    nc = _NC_CACHE

    in_maps = []
    for core in range(8):
        b, g = core // 2, core % 2
        perm = list(range(g, NH, 2))  # slot s -> original head 2s+g
        core_slopes = slopes[perm]

        qaug1 = (-core_slopes[:, None] * ii[None, :]).astype(ml_dtypes.bfloat16)
        qaugb = np.ascontiguousarray(np.broadcast_to(qaug1[:, None, :], (8, NHC, T)))
        kaugb = np.zeros((8, NHC, T), ml_dtypes.bfloat16)
        for h in range(NHC):
            kaugb[h, h, :] = ml_dtypes.bfloat16(1.0)
        biasj = np.zeros((P, NHC, NJB), np.float32)
        for h in range(NHC):
            for jb in range(NJB):
                biasj[:, h, jb] = (core_slopes[h] * (128 * jb + pp)).astype(np.float32)

        wq_g = np.concatenate([Wq[:, 64 * h : 64 * h + 64] for h in perm], axis=1)
        wk_g = np.concatenate([Wk[:, 64 * h : 64 * h + 64] for h in perm], axis=1)
        wv_g = np.concatenate([Wv[:, 64 * h : 64 * h + 64] for h in perm], axis=1)
        wo_g = np.concatenate([Wo[64 * h : 64 * h + 64, :] for h in perm], axis=0)

        in_maps.append(
            {
                "xT": np.ascontiguousarray(x[b].T),
                "wq": np.ascontiguousarray(wq_g) * np.float32(0.125),
                "wk": np.ascontiguousarray(wk_g),
                "wv": np.ascontiguousarray(wv_g),
                "wo": np.ascontiguousarray(wo_g),
                "qaugb": qaugb,
                "kaugb": kaugb,
                "biasj": biasj,
            }
        )

    res = run_bass_kernel_spmd(nc, in_maps, list(range(8)))
    LAST_RESULTS = res
    out = np.empty((B, T, C), dtype=np.float32)
    for b in range(B):
        out[b] = res.results[2 * b]["y"] + res.results[2 * b + 1]["y"]
    return out
